# revision 1
# baseline (speedup 1.0000x reference)
"""Trainium2 Bass kernel for DropChannel (topk channel masking).

Math (per sample):
    score_c = mean_hw x[hw, c]                       (only sums needed; 1/HW cancels)
    lk_c    = ln(r_c) * (1 / S_c)                    (log of key r**(1/score); order-preserving)
    gcnt_i  = #{c : lk_c > lk_i}                     (strictly-greater count)
    sel_i   = gcnt_i < C - M                         (identical to thr = sort(key)[C-M]; sel = key >= thr,
                                                      including tie behaviour)
    alpha   = sum(S) / sum(S * sel)
    out     = x * (sel & (u < P)) * alpha

Sharding: pure data parallel, N=32 samples -> 8 cores x 4 samples.

Per-core schedule (4 samples, each [4096, 1024] f32):
  - load all 32 x-tiles [128hw, 1024c] of a sample into SBUF and keep them
    (x is read from HBM exactly once: 64 MiB read + 64 MiB write per core)
  - column sums via PE matmul-with-ones accumulating into PSUM [1, 1024]
  - greater-counts: 2x-mode DVE compares of broadcast-lk against
    per-partition lk scalars, summed across partitions by full-rate bf16
    PE ones-matmuls (exact 0/1 integer arithmetic) -> gcnt in row layout;
    pipelined in 512-channel halves
  - mask row built on one partition, replicated via gpsimd.partition_broadcast
  - in-place multiply of the cached x tiles; stores + mid-chain DMAs ride
    the ACT HWDGE queue so they never wait behind prefetch loads on qSP
"""

import numpy as np
from contextlib import ExitStack

import concourse.bacc as bacc
import concourse.tile as tile
from concourse import mybir
from concourse.bass_utils import run_bass_kernel_spmd

N, HW, C = 32, 4096, 1024
NCORES = 8
NS = N // NCORES          # samples per core
P = 128                   # partitions
CK = C // P               # 8 channels per partition in (p k) layout
NKEEP = C - int(0.5 * C)  # gcnt threshold: keep rows with gcnt < 512
PKEEP = 0.9
HALF = 512                # matmul free-dim limit (one PSUM bank)

f32 = mybir.dt.float32
bf16 = mybir.dt.bfloat16
ALU = mybir.AluOpType
ACTF = mybir.ActivationFunctionType
AXIS = mybir.AxisListType

# fp32 matmuls stream at 1/4 rate on the PE; the 0/1 comparison tiles are
# emitted as bf16 instead (exact for 0/1, full-rate matmul, half the SBUF
# write traffic). The score sums must stay fp32: the selection boundary
# sits only ~2e-5 (relative) away from the threshold.


def emit(tc, o, x, r, u, ns, hw, xbufs):
    nc = tc.nc
    nt = hw // P
    xt = x.rearrange("s (t p) c -> s t p c", p=P)
    ot = o.rearrange("s (t p) c -> s t p c", p=P)
    rck = r.rearrange("s (p k) -> s p k", k=CK)

    with ExitStack() as ctx:
        xpool = ctx.enter_context(tc.tile_pool(name="xpool", bufs=xbufs))
        tqpool = ctx.enter_context(tc.tile_pool(name="tqpool", bufs=3))
        bcpool = ctx.enter_context(tc.tile_pool(name="bcpool", bufs=2))
        rows = ctx.enter_context(tc.tile_pool(name="rows", bufs=2))
        consts = ctx.enter_context(tc.tile_pool(name="consts", bufs=1))
        ps_s = ctx.enter_context(tc.tile_pool(name="ps_s", bufs=2, space="PSUM"))
        ps_g = ctx.enter_context(tc.tile_pool(name="ps_g", bufs=2, space="PSUM"))

        ones_col = consts.tile([P, 1], f32)
        nc.vector.memset(ones_col, 1.0)
        ones_b = consts.tile([P, 1], bf16)
        nc.vector.memset(ones_b, 1.0)

        for s in range(ns):
            # precompute pieces that do not depend on x (overlap with loads):
            # ln(r) in (p k) layout, and the bernoulli gate row (u < PKEEP)
            lnr_cols = rows.tile([P, CK], f32, tag="lnr_cols")
            nc.scalar.dma_start(out=lnr_cols, in_=rck[s])
            nc.scalar.activation(lnr_cols, lnr_cols, ACTF.Ln)
            rng_row = rows.tile([1, C], f32, tag="rng_row")
            nc.scalar.dma_start(out=rng_row, in_=u[s:s + 1, :])
            nc.vector.tensor_scalar(rng_row, rng_row, PKEEP, None, op0=ALU.is_lt)

            # ---- pass 1: load tiles; pair-add on DVE halves the fp32 PE
            # work (and matches the reference's pairwise summation better)
            ps_score = ps_s.tile([1, C], f32, tag="ps_score")
            xts = []
            for t in range(nt):
                xtile = xpool.tile([P, C], f32, tag="xt")
                # sample 0 has no store traffic yet: split its loads across
                # both HWDGE queues to double the trigger issue rate
                if s == 0 and t % 2 == 1:
                    nc.scalar.dma_start(out=xtile, in_=xt[s, t])
                else:
                    nc.sync.dma_start(out=xtile, in_=xt[s, t])
                xts.append(xtile)
                if t % 2 == 1:
                    acc = tqpool.tile([P, C], f32, tag="acc", bufs=2)
                    nc.vector.tensor_add(acc, xts[t - 1], xts[t])
                    for h in range(2):
                        nc.tensor.matmul(
                            ps_score[:, h * HALF:(h + 1) * HALF],
                            lhsT=ones_col,
                            rhs=acc[:, h * HALF:(h + 1) * HALF],
                            start=(t == 1),
                            stop=(t == nt - 1),
                        )

            # ---- mid: selection mask ----
            # column sums into (p k) layout, reciprocal + logkey there (128-way)
            s_row = rows.tile([1, C], f32, tag="s_row", bufs=1)
            nc.scalar.copy(s_row[:, 0:HALF], ps_score[:, 0:HALF])
            nc.vector.tensor_copy(s_row[:, HALF:], ps_score[:, HALF:])
            s_cols = rows.tile([P, CK], f32, tag="s_cols", bufs=1)
            nc.scalar.dma_start(out=s_cols, in_=s_row)
            recip_cols = rows.tile([P, CK], f32, tag="recip_cols", bufs=1)
            nc.vector.reciprocal(recip_cols, s_cols)
            lk_cols = rows.tile([P, CK], f32, tag="lk_cols", bufs=1)
            nc.vector.tensor_mul(lk_cols, lnr_cols, recip_cols)
            # gcnt_row[i] = #{c : lk_c > lk_i}: 2x-mode compares feeding
            # full-rate bf16 ones-matmuls (0/1 values are exact in bf16).
            # Processed in 512-channel halves so half 1's compares overlap
            # half 0's mask + broadcast.
            lk_row = rows.tile([1, C], f32, tag="lk_row", bufs=1)
            b_bc = bcpool.tile([P, C], f32, tag="b_bc", bufs=1)
            ps_gcnt = ps_g.tile([1, C], f32, tag="ps_gcnt", bufs=1)
            mask_row = rows.tile([1, C], f32, tag="mask_row", bufs=1)
            mask_bc = bcpool.tile([P, C], f32, tag="mask_bc", bufs=1)
            for h in range(2):
                sl = slice(h * HALF, (h + 1) * HALF)
                nc.scalar.dma_start(
                    out=lk_row[:, sl], in_=lk_cols[h * 64:(h + 1) * 64, :]
                )
                nc.gpsimd.partition_broadcast(b_bc[:, sl], lk_row[:, sl])
                for q in range(CK):
                    tq = tqpool.tile([P, HALF], bf16, tag="tq", bufs=4)
                    nc.vector.tensor_scalar(
                        tq, b_bc[:, sl], lk_cols[:, q:q + 1], None, op0=ALU.is_lt
                    )
                    nc.tensor.matmul(
                        ps_gcnt[:, sl],
                        lhsT=ones_b,
                        rhs=tq,
                        start=(q == 0),
                        stop=(q == CK - 1),
                    )
                nc.vector.scalar_tensor_tensor(
                    mask_row[:, sl], ps_gcnt[:, sl], float(NKEEP), rng_row[:, sl],
                    op0=ALU.is_lt, op1=ALU.mult,
                )
                nc.gpsimd.partition_broadcast(mask_bc[:, sl], mask_row[:, sl])

            # alpha = sum(S) / sum(S * sel); rng_row doubles as scratch out
            stats = rows.tile([1, 3], f32, tag="stats", bufs=1)
            nc.vector.scalar_tensor_tensor(
                rng_row, ps_gcnt, float(NKEEP), s_row,
                op0=ALU.is_lt, op1=ALU.mult, accum_out=stats[:, 0:1],
            )
            nc.vector.tensor_reduce(stats[:, 1:2], s_row, axis=AXIS.X, op=ALU.add)
            nc.vector.reciprocal(stats[:, 2:3], stats[:, 0:1])
            nc.vector.tensor_scalar(
                stats[:, 2:3], stats[:, 2:3], stats[:, 1:2], None, op0=ALU.mult
            )
            alpha_pp = rows.tile([P, 1], f32, tag="alpha_pp", bufs=1)
            nc.gpsimd.partition_broadcast(alpha_pp, stats[:, 2:3])

            # ---- pass 2: out = (x * alpha) * mask, in place, then store ----
            for t in range(nt):
                nc.vector.scalar_tensor_tensor(
                    xts[t], xts[t], alpha_pp, mask_bc,
                    op0=ALU.mult, op1=ALU.mult,
                )
                nc.scalar.dma_start(out=ot[s, t], in_=xts[t])


def build_nc(ns=NS, hw=HW, xbufs=41):
    nc = bacc.Bacc(
        "TRN2", target_bir_lowering=False, debug=False, num_devices=NCORES
    )
    x = nc.dram_tensor("x", [ns, hw, C], f32, kind="ExternalInput").ap()
    r = nc.dram_tensor("r", [ns, C], f32, kind="ExternalInput").ap()
    u = nc.dram_tensor("u", [ns, C], f32, kind="ExternalInput").ap()
    o = nc.dram_tensor("o", [ns, hw, C], f32, kind="ExternalOutput").ap()
    with tile.TileContext(nc) as tc:
        emit(tc, o, x, r, u, ns, hw, xbufs)
    nc.compile()
    return nc


_cached_nc = None


def kernel(x, r, u):
    global _cached_nc
    if _cached_nc is None:
        _cached_nc = build_nc()
    in_maps = [
        {
            "x": np.ascontiguousarray(x[i * NS:(i + 1) * NS], dtype=np.float32),
            "r": np.ascontiguousarray(r[i * NS:(i + 1) * NS], dtype=np.float32),
            "u": np.ascontiguousarray(u[i * NS:(i + 1) * NS], dtype=np.float32),
        }
        for i in range(NCORES)
    ]
    res = run_bass_kernel_spmd(_cached_nc, in_maps, list(range(NCORES))).results
    return np.concatenate([res[i]["o"] for i in range(NCORES)], axis=0)



# revision 2
# speedup vs baseline: 1.0700x; 1.0700x over previous
"""Trainium2 Bass kernel for DropChannel (topk channel masking).

Math (per sample):
    score_c = mean_hw x[hw, c]                       (only sums needed; 1/HW cancels)
    lk_c    = ln(r_c) * (1 / S_c)                    (log of key r**(1/score); order-preserving)
    gcnt_i  = #{c : lk_c > lk_i}                     (strictly-greater count)
    sel_i   = gcnt_i < C - M                         (identical to thr = sort(key)[C-M]; sel = key >= thr,
                                                      including tie behaviour)
    alpha   = sum(S) / sum(S * sel)
    out     = x * (sel & (u < P)) * alpha

Sharding: pure data parallel, N=32 samples -> 8 cores x 4 samples.

Per-core schedule (4 samples, each [4096, 1024] f32), restructured so the
HBM stream never stalls on the serial mask chain:
  - x tiles are [128, 2048] (two hw rows per partition line -> 8 KiB
    contiguous HBM per partition, 2x the descriptor size of a [128, 1024]
    tile; reads were descriptor-latency-bound at 4 KiB)
  - each tile lands in a small f32 staging pool; the score matmuls (exact
    fp32, required for bit-exact top-k selection) read the staged tile,
    and a DVE copy casts it to a bf16 resident tile; staging recycles
  - bf16 residency halves SBUF: TWO full samples fit resident, so sample
    s+1's loads proceed at full rate while sample s's mask chain runs --
    the ~15us serial chain no longer leaves HBM idle
  - pass 2 multiplies the bf16 tiles in place (mask 0/1 bf16 exact, alpha
    applied as an f32 per-partition scalar) and stores via SWDGE DMA with
    an inline bf16->f32 cast (output precision ~4e-3 relative, gate 2e-2)
  - queue separation: x loads on qSP (sync), mid-chain small DMAs on qACT
    (scalar), stores on SWDGE (gpsimd) -- no head-of-line blocking of
    prefetch loads behind chain-dependent triggers
"""

import numpy as np
from contextlib import ExitStack

import concourse.bacc as bacc
import concourse.tile as tile
from concourse import mybir
from concourse.bass_utils import run_bass_kernel_spmd

N, HW, C = 32, 4096, 1024
NCORES = 8
NS = N // NCORES          # samples per core
P = 128                   # partitions
CK = C // P               # 8 channels per partition in (p k) layout
NKEEP = C - int(0.5 * C)  # gcnt threshold: keep rows with gcnt < 512
PKEEP = 0.9
HALF = 512                # matmul free-dim limit (one PSUM bank)
TWO = 2                   # hw rows per partition line
FREE = TWO * C            # 2048 free dim per tile
NT = HW // (P * TWO)      # 16 tiles per sample

f32 = mybir.dt.float32
bf16 = mybir.dt.bfloat16
ALU = mybir.AluOpType
ACTF = mybir.ActivationFunctionType
AXIS = mybir.AxisListType

# The score sums must stay fp32: the selection boundary sits only ~2e-5
# (relative) away from the threshold, so scores are matmul'd from the f32
# staging tiles before the bf16 cast. The 0/1 comparison tiles are bf16
# (exact for 0/1, full-rate PE matmul).


def emit(tc, o, x, r, u, ns, hw):
    nc = tc.nc
    nt = hw // (P * TWO)
    xt = x.rearrange("s (t p two) c -> s t p (two c)", p=P, two=TWO)
    ot = o.rearrange("s (t p two) c -> s t p (two c)", p=P, two=TWO)
    rck = r.rearrange("s (p k) -> s p k", k=CK)

    with ExitStack() as ctx:
        stage = ctx.enter_context(tc.tile_pool(name="stage", bufs=3))
        xpool = ctx.enter_context(tc.tile_pool(name="xpool", bufs=2 * NT))
        tqpool = ctx.enter_context(tc.tile_pool(name="tqpool", bufs=4))
        bcpool = ctx.enter_context(tc.tile_pool(name="bcpool", bufs=2))
        rows = ctx.enter_context(tc.tile_pool(name="rows", bufs=2))
        consts = ctx.enter_context(tc.tile_pool(name="consts", bufs=1))
        ps_s = ctx.enter_context(tc.tile_pool(name="ps_s", bufs=2, space="PSUM"))
        ps_g = ctx.enter_context(tc.tile_pool(name="ps_g", bufs=2, space="PSUM"))

        ones_col = consts.tile([P, 1], f32)
        nc.vector.memset(ones_col, 1.0)
        ones_b = consts.tile([P, 1], bf16)
        nc.vector.memset(ones_b, 1.0)

        for s in range(ns):
            # precompute pieces that do not depend on x (overlap with loads):
            # ln(r) in (p k) layout, and the bernoulli gate row (u < PKEEP)
            lnr_cols = rows.tile([P, CK], f32, tag="lnr_cols")
            nc.scalar.dma_start(out=lnr_cols, in_=rck[s])
            nc.scalar.activation(lnr_cols, lnr_cols, ACTF.Ln)
            rng_row = rows.tile([1, C], f32, tag="rng_row")
            nc.scalar.dma_start(out=rng_row, in_=u[s:s + 1, :])
            nc.vector.tensor_scalar(rng_row, rng_row, PKEEP, None, op0=ALU.is_lt)

            # ---- pass 1: stage f32 tiles, matmul scores, cast to bf16 ----
            ps_score = ps_s.tile([1, C], f32, tag="ps_score")
            xts = []
            for t in range(nt):
                xstage = stage.tile([P, FREE], f32, tag="xstage")
                nc.sync.dma_start(out=xstage, in_=xt[s, t])
                xres = xpool.tile([P, FREE], bf16, tag="xres")
                nc.vector.tensor_copy(xres, xstage)
                for g in range(TWO):
                    for h in range(2):
                        nc.tensor.matmul(
                            ps_score[:, h * HALF:(h + 1) * HALF],
                            lhsT=ones_col,
                            rhs=xstage[:, g * C + h * HALF:g * C + (h + 1) * HALF],
                            start=(t == 0 and g == 0),
                            stop=(t == nt - 1 and g == TWO - 1),
                        )
                xts.append(xres)

            # ---- mid: selection mask ----
            # column sums into (p k) layout, reciprocal + logkey there (128-way)
            s_row = rows.tile([1, C], f32, tag="s_row", bufs=2)
            nc.scalar.copy(s_row[:, 0:HALF], ps_score[:, 0:HALF])
            nc.vector.tensor_copy(s_row[:, HALF:], ps_score[:, HALF:])
            s_cols = rows.tile([P, CK], f32, tag="s_cols", bufs=2)
            nc.scalar.dma_start(out=s_cols, in_=s_row)
            recip_cols = rows.tile([P, CK], f32, tag="recip_cols", bufs=2)
            nc.vector.reciprocal(recip_cols, s_cols)
            lk_cols = rows.tile([P, CK], f32, tag="lk_cols", bufs=2)
            nc.vector.tensor_mul(lk_cols, lnr_cols, recip_cols)
            # gcnt_row[i] = #{c : lk_c > lk_i}: DVE compares feeding full-rate
            # bf16 ones-matmuls (0/1 values are exact in bf16). Processed in
            # 512-channel halves so half 1's compares overlap half 0's mask.
            lk_row = rows.tile([1, C], f32, tag="lk_row", bufs=2)
            b_bc = bcpool.tile([P, C], f32, tag="b_bc", bufs=1)
            ps_gcnt = ps_g.tile([1, C], f32, tag="ps_gcnt")
            mask_row = rows.tile([1, C], bf16, tag="mask_row", bufs=2)
            mask_bc = bcpool.tile([P, FREE], bf16, tag="mask_bc", bufs=2)
            for h in range(2):
                sl = slice(h * HALF, (h + 1) * HALF)
                nc.scalar.dma_start(
                    out=lk_row[:, sl], in_=lk_cols[h * 64:(h + 1) * 64, :]
                )
                nc.gpsimd.partition_broadcast(b_bc[:, sl], lk_row[:, sl])
                for q in range(CK):
                    tq = tqpool.tile([P, HALF], bf16, tag="tq", bufs=4)
                    nc.vector.tensor_scalar(
                        tq, b_bc[:, sl], lk_cols[:, q:q + 1], None, op0=ALU.is_lt
                    )
                    nc.tensor.matmul(
                        ps_gcnt[:, sl],
                        lhsT=ones_b,
                        rhs=tq,
                        start=(q == 0),
                        stop=(q == CK - 1),
                    )
                nc.vector.scalar_tensor_tensor(
                    mask_row[:, sl], ps_gcnt[:, sl], float(NKEEP), rng_row[:, sl],
                    op0=ALU.is_lt, op1=ALU.mult,
                )
                for g in range(TWO):
                    nc.gpsimd.partition_broadcast(
                        mask_bc[:, g * C + h * HALF:g * C + (h + 1) * HALF],
                        mask_row[:, sl],
                    )

            # alpha = sum(S) / sum(S * sel)
            scratch = rows.tile([1, C], f32, tag="scratch", bufs=2)
            stats = rows.tile([1, 3], f32, tag="stats", bufs=2)
            nc.vector.scalar_tensor_tensor(
                scratch, ps_gcnt, float(NKEEP), s_row,
                op0=ALU.is_lt, op1=ALU.mult, accum_out=stats[:, 0:1],
            )
            nc.vector.tensor_reduce(stats[:, 1:2], s_row, axis=AXIS.X, op=ALU.add)
            nc.vector.reciprocal(stats[:, 2:3], stats[:, 0:1])
            nc.vector.tensor_scalar(
                stats[:, 2:3], stats[:, 2:3], stats[:, 1:2], None, op0=ALU.mult
            )
            alpha_pp = rows.tile([P, 1], f32, tag="alpha_pp", bufs=2)
            nc.gpsimd.partition_broadcast(alpha_pp, stats[:, 2:3])

            # ---- pass 2: out = (x * alpha) * mask in bf16, SWDGE cast-store ----
            for t in range(nt):
                nc.vector.scalar_tensor_tensor(
                    xts[t], xts[t], alpha_pp, mask_bc,
                    op0=ALU.mult, op1=ALU.mult,
                )
                nc.gpsimd.dma_start(out=ot[s, t], in_=xts[t])


def build_nc(ns=NS, hw=HW):
    nc = bacc.Bacc(
        "TRN2", target_bir_lowering=False, debug=False, num_devices=NCORES
    )
    x = nc.dram_tensor("x", [ns, hw, C], f32, kind="ExternalInput").ap()
    r = nc.dram_tensor("r", [ns, C], f32, kind="ExternalInput").ap()
    u = nc.dram_tensor("u", [ns, C], f32, kind="ExternalInput").ap()
    o = nc.dram_tensor("o", [ns, hw, C], f32, kind="ExternalOutput").ap()
    with tile.TileContext(nc) as tc:
        emit(tc, o, x, r, u, ns, hw)
    nc.compile()
    return nc


_cached_nc = None


def kernel(x, r, u):
    global _cached_nc
    if _cached_nc is None:
        _cached_nc = build_nc()
    in_maps = [
        {
            "x": np.ascontiguousarray(x[i * NS:(i + 1) * NS], dtype=np.float32),
            "r": np.ascontiguousarray(r[i * NS:(i + 1) * NS], dtype=np.float32),
            "u": np.ascontiguousarray(u[i * NS:(i + 1) * NS], dtype=np.float32),
        }
        for i in range(NCORES)
    ]
    res = run_bass_kernel_spmd(_cached_nc, in_maps, list(range(NCORES))).results
    return np.concatenate([res[i]["o"] for i in range(NCORES)], axis=0)
